# revision 24
# baseline (speedup 1.0000x reference)
"""Trainium2 Bass kernel for nn_AE_spikes (spiking autoencoder, 16-step scan).

Data-parallel over 8 NeuronCores: batch 16384 -> 2048 rows/core.

V2 design (vs v1 baseline at ~1031us):
  - NT=256 batch tiles -> ALL membranes PSUM-resident:
      pv123 [128,1024] (2 banks, bufs=2): v1, v2, v3 (+256 spare)
      pv4   [128,2048] (4 banks, bufs=1): v4 chunks 0..6 at cols 0..1791
    Membrane updates are matmul-only (weights, -I resets); no DVE/ACT
    read-modify-write of membranes at all.
  - Stateless encoder: s0_t = (((t+1)*f) mod 1) < f  (2 DVE ops/step,
    replaces v0 state + 3 passes).
  - Single fp16 weights (|W|~0.04, fp16 rel err 5e-4) instead of hi/lo.
  - 112-row feature chunks; bias injected via an extra K-row on the
    K=112 matmuls (s0/s4 tiles carry a constant-1 row at partition 112),
    so v1/v4 are *true* potentials -> constant spike threshold -> one
    wide ACT sigmoid-step over all 7 v4 chunks.
  - Output count via conservation: count4 = W4*S3 + 16*b4 - v4_final,
    with S3 = sum_t s3_t accumulated in fp16 (exact). No per-step count.
"""

import os
import sys

import numpy as np

if "/opt/trn_rl_repo" not in sys.path:
    sys.path.insert(0, "/opt/trn_rl_repo")

B = 16384
IN = 784
H = 128
T = 16
NCORES = 8
BC = B // NCORES          # 2048 batch rows per core
NT = 256                  # batch-tile columns
NTILES = BC // NT         # 8
CH = 7                    # feature chunks of 112 rows
KC = 112
CW = CH * NT              # concatenated width, 1792
KAPPA = float(2 ** 30)    # sigmoid-step scale
W4LO = False              # W4 lo-correction matmuls (False: single-f16 W4,
                          # sim rel err 0.0104 vs tolerance 2e-2)

LAST_RESULT = None
_CACHE = {}


def _install_ntff_shim():
    """Make run_bass_kernel_spmd(trace=True) work in this container."""
    import types

    try:
        from antenv.axon_hooks import get_axon_ntff_profile_hook  # noqa: F401
        return
    except ImportError:
        pass
    try:
        import antenv
        from trn_agent_boot.trn_boot import _ntff_profile_via_ctypes
    except ImportError:
        return
    mod = types.ModuleType("antenv.axon_hooks")
    mod._hook = _ntff_profile_via_ctypes("/opt/axon/libaxon_pjrt.so")
    mod.set_axon_ntff_profile_hook = lambda h: setattr(mod, "_hook", h)
    mod.get_axon_ntff_profile_hook = lambda: mod._hook
    sys.modules["antenv.axon_hooks"] = mod
    antenv.axon_hooks = mod


def _build():
    import concourse.tile as tile
    from concourse import bacc, mybir
    from contextlib import ExitStack

    f32 = mybir.dt.float32
    f16 = mybir.dt.float16
    Alu = mybir.AluOpType
    ActF = mybir.ActivationFunctionType

    nc = bacc.Bacc("TRN2", target_bir_lowering=False, debug=False)

    fT_d = nc.dram_tensor("fT", [IN, BC], f32, kind="ExternalInput").ap()
    # Weights as fp16 hi/lo splits (w = hi + lo exactly enough; products
    # with binary spikes are then fp32-exact in PSUM). The spiking dynamics
    # are chaotic at fp16-single weight precision (rel err 0.10) - exact
    # products are required.
    # w1s: [113, 128] per chunk, concatenated along free dim: chunk c at
    # cols [c*H, (c+1)*H). Row 112 of chunk 0 is b1 (hi/lo split across
    # the two stationaries); rows 112 of chunks 1..6 are zero.
    w1sh_d = nc.dram_tensor("w1sh", [KC + 1, CH * H], f16, kind="ExternalInput").ap()
    w1sl_d = nc.dram_tensor("w1sl", [KC + 1, CH * H], f16, kind="ExternalInput").ap()
    w2h_d = nc.dram_tensor("w2h", [H, H], f16, kind="ExternalInput").ap()
    w2l_d = nc.dram_tensor("w2l", [H, H], f16, kind="ExternalInput").ap()
    w3h_d = nc.dram_tensor("w3h", [H, H], f16, kind="ExternalInput").ap()
    w3l_d = nc.dram_tensor("w3l", [H, H], f16, kind="ExternalInput").ap()
    w4Th_d = nc.dram_tensor("w4Th", [H, IN], f16, kind="ExternalInput").ap()
    w4Tl_d = nc.dram_tensor("w4Tl", [H, IN], f16, kind="ExternalInput").ap()
    nw4Th_d = nc.dram_tensor("nw4Th", [H, IN], f16, kind="ExternalInput").ap()
    nw4Tl_d = nc.dram_tensor("nw4Tl", [H, IN], f16, kind="ExternalInput").ap()
    # rst4: [113, 112] per chunk (cols c*KC..): rows 0..111 = -I, row 112 = b4 chunk
    rst4_d = nc.dram_tensor("rst4", [KC + 1, IN], f16, kind="ExternalInput").ap()
    negI_d = nc.dram_tensor("negI", [H, H], f16, kind="ExternalInput").ap()
    thh_d = nc.dram_tensor("thh", [H, 2 * T], f32, kind="ExternalInput").ap()
    b4v_d = nc.dram_tensor("b4v", [KC, CH], f32, kind="ExternalInput").ap()
    out_d = nc.dram_tensor("outT", [IN, BC], f32, kind="ExternalOutput").ap()

    with tile.TileContext(nc) as tc:
        with ExitStack() as ctx:
            wp = ctx.enter_context(tc.tile_pool(name="weights", bufs=1))
            fp = ctx.enter_context(tc.tile_pool(name="feat", bufs=2))
            cpp = ctx.enter_context(tc.tile_pool(name="cp1p", bufs=2))
            shp = ctx.enter_context(tc.tile_pool(name="shid", bufs=3))
            s3ap = ctx.enter_context(tc.tile_pool(name="s3ap", bufs=2))
            outp = ctx.enter_context(tc.tile_pool(name="outp", bufs=2))
            s0p = ctx.enter_context(tc.tile_pool(name="s0p", bufs=1))
            s4p = ctx.enter_context(tc.tile_pool(name="s4p", bufs=1))
            pv123p = ctx.enter_context(
                tc.tile_pool(name="pv123", bufs=2, space="PSUM"))
            pv4p = ctx.enter_context(
                tc.tile_pool(name="pv4", bufs=1, space="PSUM"))

            # ---- load weights / tables once ----
            def wload(name, dram, shape):
                tl = wp.tile(shape, f16, tag=name, name=name)
                nc.sync.dma_start(tl[:], dram[:])
                return tl

            w1sh = wload("w1sh", w1sh_d, [KC + 1, CH * H])
            w1sl = wload("w1sl", w1sl_d, [KC + 1, CH * H])
            w2h = wload("w2h", w2h_d, [H, H])
            w2l = wload("w2l", w2l_d, [H, H])
            w3h = wload("w3h", w3h_d, [H, H])
            w3l = wload("w3l", w3l_d, [H, H])
            w4Th = wload("w4Th", w4Th_d, [H, IN])
            w4Tl = wload("w4Tl", w4Tl_d, [H, IN])
            nw4Th = wload("nw4Th", nw4Th_d, [H, IN])
            nw4Tl = wload("nw4Tl", nw4Tl_d, [H, IN])
            rst4 = wp.tile([KC + 1, IN], f16, tag="rst4")
            nc.sync.dma_start(rst4[:], rst4_d[:])
            negI = wp.tile([H, H], f16, tag="negI")
            nc.sync.dma_start(negI[:], negI_d[:])
            thh = wp.tile([H, 2 * T], f32, tag="thh")
            nc.sync.dma_start(thh[:], thh_d[:])
            b4v = wp.tile([KC, CH], f32, tag="b4v")
            nc.sync.dma_start(b4v[:], b4v_d[:])
            nk1 = wp.tile([H, 1], f32, tag="nk1")
            nc.gpsimd.memset(nk1[:], -KAPPA)

            # persistent spike buffers (rotating by step mod 3), with the
            # constant-1 row at partition 112 (bias row for K=113 matmuls)
            s0b = []
            s4b = []
            for i in range(3):
                # ones row lives at partition 112; memset must start at a
                # quadrant boundary, so fill [96:113] then let the per-step
                # writes to [0:112] overwrite the data rows.
                t0 = s0p.tile([KC + 1, CW], f16, tag=f"s0_{i}", name=f"s0_{i}")
                nc.gpsimd.memset(t0[96:KC + 1, :], 1.0)
                s0b.append(t0)
                t4 = s4p.tile([KC + 1, CW], f16, tag=f"s4_{i}", name=f"s4_{i}")
                nc.gpsimd.memset(t4[96:KC + 1, :], 1.0)
                s4b.append(t4)

            def enc(fTt, cp1, t, dst):
                """s0_t = ((t+1)*f >= cp1); cp1 += s0.

                cp1 = (encoder spike count so far) + 1, kept in f16 (exact,
                <= 17). Equivalent to floor((t+1)f) - floor(tf) = 1.
                """
                nc.vector.scalar_tensor_tensor(
                    dst[0:KC, :], fTt[:], float(t + 1), cp1[:],
                    Alu.mult, Alu.is_ge)
                nc.vector.tensor_tensor(cp1[:], cp1[:], dst[0:KC, :], Alu.add)

            for b in range(NTILES):
                c0 = b * NT
                fTt = fp.tile([KC, CW], f32, tag="fT")
                for c in range(CH):
                    nc.sync.dma_start(
                        fTt[:, c * NT:(c + 1) * NT],
                        fT_d[KC * c:KC * (c + 1), c0:c0 + NT])
                pv123 = pv123p.tile([H, 1024], f32, tag="pv123", name=f"pv123_{b}")
                v1 = pv123[:, 0:NT]
                v2 = pv123[:, NT:2 * NT]
                v3 = pv123[:, 2 * NT:3 * NT]
                pv4 = pv4p.tile([H, 2048], f32, tag="pv4", name=f"pv4_{b}")
                s3a = s3ap.tile([H, NT], f16, tag="s3a", name=f"s3a_{b}")
                nc.vector.memset(s3a[:], 0.0)
                # s4[-1] := 0 (its -I contribution at t=0 must vanish; the
                # b4 bias row still fires, initializing v4 to b4)
                nc.gpsimd.memset(s4b[2][0:KC, :], 0.0)
                cp1 = cpp.tile([KC, CW], f16, tag="cp1", name=f"cp1_{b}")
                nc.gpsimd.memset(cp1[:], 1.0)

                def emit_L1(t, s0):
                    """15 weight matmuls of layer 1 (hi/lo x 7 chunks +
                    bias row on chunk 0)."""
                    for c in range(CH):
                        kc = KC + 1 if c == 0 else KC
                        rhs = s0[0:kc, c * NT:(c + 1) * NT]
                        nc.tensor.matmul(
                            v1, w1sh[0:kc, c * H:(c + 1) * H], rhs,
                            start=(t == 0 and c == 0), stop=False,
                            skip_group_check=True)
                        nc.tensor.matmul(
                            v1, w1sl[0:kc, c * H:(c + 1) * H], rhs,
                            start=False,
                            stop=(t == T - 1 and c == CH - 1),
                            skip_group_check=True)

                def emit_rst4(t, s4_prev, chunks):
                    for c in chunks:
                        dst = pv4[0:KC, c * NT:(c + 1) * NT]
                        nc.tensor.matmul(
                            dst, rst4[:, c * KC:(c + 1) * KC],
                            s4_prev[:, c * NT:(c + 1) * NT],
                            start=False, stop=False,
                            skip_group_check=True)

                # Software-pipelined emission: L1 of step t is emitted during
                # step t-1 so each engine's queue always has ready work to
                # fill the spike-latency gaps (rst4 fills the s1 wait, the
                # immediate post-read resets fill the s2/s3 waits).
                enc(fTt, cp1, 0, s0b[0])
                emit_L1(0, s0b[0])

                for t in range(T):
                    s0n = s0b[(t + 1) % 3]
                    s4_prev = s4b[(t - 1) % 3]
                    s4 = s4b[t % 3]
                    # spike of layer 1 (L1(t) was emitted in step t-1;
                    # s4(t-1) ran on ACT during that block, so ACT is free)
                    s1 = shp.tile([H, NT], f16, tag="s1")
                    nc.scalar.activation(s1[:], v1, ActF.Sigmoid,
                                         bias=nk1[:], scale=KAPPA)

                    # -------- layer 2 (v2 shares v1's bank: no start) ----
                    nc.tensor.matmul(v2, w2h[:], s1[:], start=False,
                                     stop=False, skip_group_check=True)
                    nc.tensor.matmul(v2, w2l[:], s1[:], start=False,
                                     stop=(t == T - 1), skip_group_check=True)
                    # reset v1 right after its spike was read; fills s2 wait
                    if t < T - 1:
                        nc.tensor.matmul(v1, negI[:], s1[:], start=False,
                                         stop=False, skip_group_check=True)
                    s2 = shp.tile([H, NT], f16, tag="s2")
                    nc.scalar.activation(s2[:], v2, ActF.Sigmoid,
                                         bias=thh[:, t:t + 1], scale=KAPPA)

                    # -------- layer 3 --------
                    nc.tensor.matmul(v3, w3h[:], s2[:], start=(t == 0),
                                     stop=False, skip_group_check=True)
                    nc.tensor.matmul(v3, w3l[:], s2[:], start=False,
                                     stop=(t == T - 1), skip_group_check=True)
                    if t < T - 1:
                        nc.tensor.matmul(v2, negI[:], s2[:], start=False,
                                         stop=False, skip_group_check=True)
                    s3 = shp.tile([H, NT], f16, tag="s3")
                    nc.scalar.activation(s3[:], v3, ActF.Sigmoid,
                                         bias=thh[:, T + t:T + t + 1], scale=KAPPA)

                    # encoder for next step (independent; fills DVE)
                    if t + 1 < T:
                        enc(fTt, cp1, t + 1, s0n)

                    # -------- layer 4 weight matmuls, then resets --------
                    # pv4 bank epochs open on the hi-matmul of even chunks
                    # at t=0 (first write of each bank this tile)
                    for c in range(CH):
                        dst = pv4[0:KC, c * NT:(c + 1) * NT]
                        nc.tensor.matmul(
                            dst, w4Th[:, c * KC:(c + 1) * KC], s3[:],
                            start=(t == 0 and c % 2 == 0), stop=False,
                            skip_group_check=True)
                        if W4LO:
                            nc.tensor.matmul(
                                dst, w4Tl[:, c * KC:(c + 1) * KC], s3[:],
                                start=False, stop=False, skip_group_check=True)
                    # rst4 AFTER the weight matmuls: s4(t-1) then has until
                    # mid-slot to complete -> the wide s4 ACT op is off the
                    # critical chain (it runs under the L1(t+1) block).
                    emit_rst4(t, s4_prev, range(CH))
                    if t < T - 1:
                        nc.tensor.matmul(v3, negI[:], s3[:], start=False,
                                         stop=False, skip_group_check=True)
                    nc.scalar.activation(s4[0:KC, :], pv4[0:KC, 0:CW],
                                         ActF.Sigmoid, bias=nk1[0:KC, :],
                                         scale=KAPPA)
                    # S3 accumulation (fp16 exact, max 16)
                    nc.vector.tensor_tensor(s3a[:], s3a[:], s3[:], Alu.add)
                    # next step's layer 1 (fills the tail of this slot)
                    if t + 1 < T:
                        emit_L1(t + 1, s0n)

                # ---- finish tile: count4 = W4*S3 + 16*b4 - v4_final ----
                # final reset (s4[15]) must be applied to v4 first
                s4_last = s4b[(T - 1) % 3]
                out = outp.tile([KC, CW], f32, tag="out")
                for c in range(CH):
                    dst = pv4[0:KC, c * NT:(c + 1) * NT]
                    nc.tensor.matmul(
                        dst, rst4[:, c * KC:(c + 1) * KC],
                        s4_last[:, c * NT:(c + 1) * NT],
                        start=False, stop=False, skip_group_check=True)
                    # note: this also adds one extra b4 (16 resets at t=1..15
                    # plus t=0 init plus this one = 17): compensated in b4v.
                    nc.tensor.matmul(
                        dst, nw4Th[:, c * KC:(c + 1) * KC], s3a[:],
                        start=False, stop=(not W4LO), skip_group_check=True)
                    if W4LO:
                        nc.tensor.matmul(
                            dst, nw4Tl[:, c * KC:(c + 1) * KC], s3a[:],
                            start=False, stop=True, skip_group_check=True)
                    # out = b4 - (v4 - W4*S3)/16 = count/16  (b4v = b4*(1+1/16)
                    # to cancel the extra bias-row add above)
                    nc.scalar.activation(
                        out[:, c * NT:(c + 1) * NT], dst, ActF.Identity,
                        bias=b4v[:, c:c + 1], scale=-1.0 / 16.0)
                for c in range(CH):
                    nc.sync.dma_start(
                        out_d[KC * c:KC * (c + 1), c0:c0 + NT],
                        out[:, c * NT:(c + 1) * NT])

    nc.compile()
    return nc


def _host_prep(inputs):
    f32 = np.float32
    f16 = np.float16
    features = np.asarray(inputs["features"], f32)
    fT = np.ascontiguousarray(features.T)  # [784, 16384]

    b1 = np.asarray(inputs["b1"], f32)
    b2 = np.asarray(inputs["b2"], f32)
    b3 = np.asarray(inputs["b3"], f32)
    b4 = np.asarray(inputs["b4"], f32)

    def split(w):
        hi = w.astype(f16)
        lo = (w - hi.astype(f32)).astype(f16)
        return hi, lo

    w1T = np.asarray(inputs["W1"], f32).T               # [784, 128]
    b1h, b1l = split(b1)
    w1Th, w1Tl = split(w1T)
    w1sh = np.zeros((KC + 1, CH * H), f16)
    w1sl = np.zeros((KC + 1, CH * H), f16)
    for c in range(CH):
        w1sh[0:KC, c * H:(c + 1) * H] = w1Th[c * KC:(c + 1) * KC, :]
        w1sl[0:KC, c * H:(c + 1) * H] = w1Tl[c * KC:(c + 1) * KC, :]
    w1sh[KC, 0:H] = b1h
    w1sl[KC, 0:H] = b1l

    w2h, w2l = split(np.ascontiguousarray(np.asarray(inputs["W2"], f32).T))
    w3h, w3l = split(np.ascontiguousarray(np.asarray(inputs["W3"], f32).T))
    w4Th, w4Tl = split(np.ascontiguousarray(np.asarray(inputs["W4"], f32).T))
    nw4Th = (-w4Th).astype(f16)
    nw4Tl = (-w4Tl).astype(f16)

    rst4 = np.zeros((KC + 1, IN), f16)
    for c in range(CH):
        blk = np.zeros((KC + 1, KC), f16)
        blk[0:KC, :] = -np.eye(KC, dtype=f16)
        blk[KC, :] = b4[c * KC:(c + 1) * KC].astype(f16)
        rst4[:, c * KC:(c + 1) * KC] = blk

    negI = (-np.eye(H)).astype(f16)

    # thresholds for layers 2,3: th[o,t] = 1 - cumsum(b)[t+1 terms], as
    # -KAPPA*th for the sigmoid bias. Iterative fp32 cumsum. v2 at step t
    # (0-based) has had (t+1) bias adds folded out.
    def cum_th(bv):
        c = np.zeros_like(bv)
        th = np.empty((bv.shape[0], T), f32)
        for t in range(T):
            c = (c + bv.astype(f32)).astype(f32)
            th[:, t] = (np.float32(1.0) - c).astype(f32)
        return th

    # layers 2/3 have no bias row: biases enter via time-varying threshold.
    # fp16 bias values to match what a bias row would have added? No - the
    # threshold path adds *exact* fp32 cumsum of fp32 b; reference adds b
    # each step in fp32. Close enough (tolerance 2e-2).
    thh = np.concatenate([cum_th(b2), cum_th(b3)], axis=1)  # [128, 32]
    nkthh = (-KAPPA * thh).astype(f32)

    # b4v: out = b4v - q/16 where q = v4T - W4*S3 and v4T includes 17
    # fp16-b4 bias-row adds (t=0 init + 15 per-step resets + final reset).
    # count = W4*S3 + 17*b4f16 - v4T_nobias... carefully:
    #   v4T = sum_t W4 s3_t + 17*b4f16 - sum_t s4_t
    #   => count = sum s4 = W4*S3 + 17*b4f16 - v4T
    #   out = count/16 = (17/16)*b4f16 - (v4T - W4*S3)/16
    b4f = b4.astype(f16).astype(f32)
    b4v = np.zeros((KC, CH), f32)
    for c in range(CH):
        b4v[:, c] = (17.0 / 16.0) * b4f[c * KC:(c + 1) * KC]

    shared = {
        "w1sh": np.ascontiguousarray(w1sh),
        "w1sl": np.ascontiguousarray(w1sl),
        "w2h": w2h, "w2l": w2l, "w3h": w3h, "w3l": w3l,
        "w4Th": w4Th, "w4Tl": w4Tl, "nw4Th": nw4Th, "nw4Tl": nw4Tl,
        "rst4": np.ascontiguousarray(rst4),
        "negI": negI,
        "thh": np.ascontiguousarray(nkthh),
        "b4v": np.ascontiguousarray(b4v),
    }
    in_maps = []
    for i in range(NCORES):
        m = dict(shared)
        m["fT"] = np.ascontiguousarray(fT[:, i * BC:(i + 1) * BC])
        in_maps.append(m)
    return in_maps


def kernel(**inputs):
    global LAST_RESULT
    if os.environ.get("BASS_TRACE"):
        _install_ntff_shim()
    from concourse.bass_utils import run_bass_kernel_spmd

    if "nc" not in _CACHE:
        _CACHE["nc"] = _build()
    nc = _CACHE["nc"]

    in_maps = _host_prep(inputs)
    kwargs = {}
    if os.environ.get("BASS_TRACE"):
        kwargs["tmpdir"] = os.environ.get("BASS_TRACE_DIR") or None
    try:
        res = run_bass_kernel_spmd(nc, in_maps, core_ids=list(range(NCORES)), **kwargs)
    except Exception:
        # transient device faults usually clear on retry
        import time

        time.sleep(2)
        res = run_bass_kernel_spmd(nc, in_maps, core_ids=list(range(NCORES)), **kwargs)
    LAST_RESULT = res

    outT = np.concatenate([res.results[i]["outT"] for i in range(NCORES)], axis=1)
    return np.ascontiguousarray(outT.T).astype(np.float32)


# revision 28
# speedup vs baseline: 1.4507x; 1.4507x over previous
"""Trainium2 Bass kernel for nn_AE_spikes (spiking autoencoder, 16-step scan).

Data-parallel over 8 NeuronCores: batch 16384 -> 2048 rows/core.

V2 design (vs v1 baseline at ~1031us):
  - NT=256 batch tiles -> ALL membranes PSUM-resident:
      pv123 [128,1024] (2 banks, bufs=2): v1, v2, v3 (+256 spare)
      pv4   [128,2048] (4 banks, bufs=1): v4 chunks 0..6 at cols 0..1791
    Membrane updates are matmul-only (weights, -I resets); no DVE/ACT
    read-modify-write of membranes at all.
  - Stateless encoder: s0_t = (((t+1)*f) mod 1) < f  (2 DVE ops/step,
    replaces v0 state + 3 passes).
  - Single fp16 weights (|W|~0.04, fp16 rel err 5e-4) instead of hi/lo.
  - 112-row feature chunks; bias injected via an extra K-row on the
    K=112 matmuls (s0/s4 tiles carry a constant-1 row at partition 112),
    so v1/v4 are *true* potentials -> constant spike threshold -> one
    wide ACT sigmoid-step over all 7 v4 chunks.
  - Output count via conservation: count4 = W4*S3 + 16*b4 - v4_final,
    with S3 = sum_t s3_t accumulated in fp16 (exact). No per-step count.
"""

import os
import sys

import numpy as np

if "/opt/trn_rl_repo" not in sys.path:
    sys.path.insert(0, "/opt/trn_rl_repo")

B = 16384
IN = 784
H = 128
T = 16
NCORES = 8
BC = B // NCORES          # 2048 batch rows per core
NT = 256                  # batch-tile columns
NTILES = BC // NT         # 8
CH = 7                    # feature chunks of 112 rows
KC = 112
CW = CH * NT              # concatenated width, 1792
KAPPA = float(2 ** 30)    # sigmoid-step scale
W4LO = False              # W4 lo-correction matmuls (False: single-f16 W4,
                          # sim rel err 0.0104 vs tolerance 2e-2)

LAST_RESULT = None
_CACHE = {}


def _install_ntff_shim():
    """Make run_bass_kernel_spmd(trace=True) work in this container."""
    import types

    try:
        from antenv.axon_hooks import get_axon_ntff_profile_hook  # noqa: F401
        return
    except ImportError:
        pass
    try:
        import antenv
        from trn_agent_boot.trn_boot import _ntff_profile_via_ctypes
    except ImportError:
        return
    mod = types.ModuleType("antenv.axon_hooks")
    mod._hook = _ntff_profile_via_ctypes("/opt/axon/libaxon_pjrt.so")
    mod.set_axon_ntff_profile_hook = lambda h: setattr(mod, "_hook", h)
    mod.get_axon_ntff_profile_hook = lambda: mod._hook
    sys.modules["antenv.axon_hooks"] = mod
    antenv.axon_hooks = mod


def _build():
    import concourse.tile as tile
    from concourse import bacc, mybir
    from contextlib import ExitStack

    f32 = mybir.dt.float32
    f16 = mybir.dt.float16
    Alu = mybir.AluOpType
    ActF = mybir.ActivationFunctionType

    nc = bacc.Bacc("TRN2", target_bir_lowering=False, debug=False)

    fT_d = nc.dram_tensor("fT", [IN, BC], f32, kind="ExternalInput").ap()
    # Weights as fp16 hi/lo splits (w = hi + lo exactly enough; products
    # with binary spikes are then fp32-exact in PSUM). The spiking dynamics
    # are chaotic at fp16-single weight precision (rel err 0.10) - exact
    # products are required.
    # w1s: [113, 128] per chunk, concatenated along free dim: chunk c at
    # cols [c*H, (c+1)*H). Row 112 of chunk 0 is b1 (hi/lo split across
    # the two stationaries); rows 112 of chunks 1..6 are zero.
    w1sh_d = nc.dram_tensor("w1sh", [KC + 1, CH * H], f16, kind="ExternalInput").ap()
    w1sl_d = nc.dram_tensor("w1sl", [KC + 1, CH * H], f16, kind="ExternalInput").ap()
    w2h_d = nc.dram_tensor("w2h", [H, H], f16, kind="ExternalInput").ap()
    w2l_d = nc.dram_tensor("w2l", [H, H], f16, kind="ExternalInput").ap()
    w3h_d = nc.dram_tensor("w3h", [H, H], f16, kind="ExternalInput").ap()
    w3l_d = nc.dram_tensor("w3l", [H, H], f16, kind="ExternalInput").ap()
    w4Th_d = nc.dram_tensor("w4Th", [H, IN], f16, kind="ExternalInput").ap()
    w4Tl_d = nc.dram_tensor("w4Tl", [H, IN], f16, kind="ExternalInput").ap()
    nw4Th_d = nc.dram_tensor("nw4Th", [H, IN], f16, kind="ExternalInput").ap()
    nw4Tl_d = nc.dram_tensor("nw4Tl", [H, IN], f16, kind="ExternalInput").ap()
    # rst4: [113, 112] per chunk (cols c*KC..): rows 0..111 = -I, row 112 = b4 chunk
    rst4_d = nc.dram_tensor("rst4", [KC + 1, IN], f16, kind="ExternalInput").ap()
    negI_d = nc.dram_tensor("negI", [H, H], f16, kind="ExternalInput").ap()
    thh_d = nc.dram_tensor("thh", [H, 2 * T], f32, kind="ExternalInput").ap()
    b4v_d = nc.dram_tensor("b4v", [KC, CH], f32, kind="ExternalInput").ap()
    out_d = nc.dram_tensor("outT", [IN, BC], f32, kind="ExternalOutput").ap()

    with tile.TileContext(nc) as tc:
        with ExitStack() as ctx:
            wp = ctx.enter_context(tc.tile_pool(name="weights", bufs=1))
            fp = ctx.enter_context(tc.tile_pool(name="feat", bufs=2))
            cpp = ctx.enter_context(tc.tile_pool(name="cp1p", bufs=2))
            shp = ctx.enter_context(tc.tile_pool(name="shid", bufs=3))
            s3ap = ctx.enter_context(tc.tile_pool(name="s3ap", bufs=2))
            outp = ctx.enter_context(tc.tile_pool(name="outp", bufs=2))
            s0p = ctx.enter_context(tc.tile_pool(name="s0p", bufs=1))
            s4p = ctx.enter_context(tc.tile_pool(name="s4p", bufs=1))
            pv123p = ctx.enter_context(
                tc.tile_pool(name="pv123", bufs=2, space="PSUM"))
            pv4p = ctx.enter_context(
                tc.tile_pool(name="pv4", bufs=1, space="PSUM"))

            # ---- load weights / tables once ----
            def wload(name, dram, shape):
                tl = wp.tile(shape, f16, tag=name, name=name)
                nc.sync.dma_start(tl[:], dram[:])
                return tl

            w1sh = wload("w1sh", w1sh_d, [KC + 1, CH * H])
            w1sl = wload("w1sl", w1sl_d, [KC + 1, CH * H])
            w2h = wload("w2h", w2h_d, [H, H])
            w2l = wload("w2l", w2l_d, [H, H])
            w3h = wload("w3h", w3h_d, [H, H])
            w3l = wload("w3l", w3l_d, [H, H])
            w4Th = wload("w4Th", w4Th_d, [H, IN])
            w4Tl = wload("w4Tl", w4Tl_d, [H, IN])
            nw4Th = wload("nw4Th", nw4Th_d, [H, IN])
            nw4Tl = wload("nw4Tl", nw4Tl_d, [H, IN])
            rst4 = wp.tile([KC + 1, IN], f16, tag="rst4")
            nc.sync.dma_start(rst4[:], rst4_d[:])
            negI = wp.tile([H, H], f16, tag="negI")
            nc.sync.dma_start(negI[:], negI_d[:])
            thh = wp.tile([H, 2 * T], f32, tag="thh")
            nc.sync.dma_start(thh[:], thh_d[:])
            b4v = wp.tile([KC, CH], f32, tag="b4v")
            nc.sync.dma_start(b4v[:], b4v_d[:])
            nk1 = wp.tile([H, 1], f32, tag="nk1")
            nc.gpsimd.memset(nk1[:], -KAPPA)

            # persistent spike buffers (rotating by step mod 3), with the
            # constant-1 row at partition 112 (bias row for K=113 matmuls)
            s0b = []
            s4b = []
            for i in range(3):
                # ones row lives at partition 112; memset must start at a
                # quadrant boundary, so fill [96:113] then let the per-step
                # writes to [0:112] overwrite the data rows.
                t0 = s0p.tile([KC + 1, CW], f16, tag=f"s0_{i}", name=f"s0_{i}")
                nc.gpsimd.memset(t0[96:KC + 1, :], 1.0)
                s0b.append(t0)
                t4 = s4p.tile([KC + 1, CW], f16, tag=f"s4_{i}", name=f"s4_{i}")
                nc.gpsimd.memset(t4[96:KC + 1, :], 1.0)
                s4b.append(t4)

            def enc(fTt, cp1, t, dst):
                """s0_t = ((t+1)*f >= cp1); cp1 += s0.

                cp1 = (encoder spike count so far) + 1, kept in f16 (exact,
                <= 17). Equivalent to floor((t+1)f) - floor(tf) = 1.
                """
                nc.vector.scalar_tensor_tensor(
                    dst[0:KC, :], fTt[:], float(t + 1), cp1[:],
                    Alu.mult, Alu.is_ge)
                nc.vector.tensor_tensor(cp1[:], cp1[:], dst[0:KC, :], Alu.add)

            for b in range(NTILES):
                c0 = b * NT
                fTt = fp.tile([KC, CW], f32, tag="fT")
                for c in range(CH):
                    nc.sync.dma_start(
                        fTt[:, c * NT:(c + 1) * NT],
                        fT_d[KC * c:KC * (c + 1), c0:c0 + NT])
                pv123 = pv123p.tile([H, 1024], f32, tag="pv123", name=f"pv123_{b}")
                v1 = pv123[:, 0:NT]
                v2 = pv123[:, NT:2 * NT]
                v3 = pv123[:, 2 * NT:3 * NT]
                pv4 = pv4p.tile([H, 2048], f32, tag="pv4", name=f"pv4_{b}")
                s3a = s3ap.tile([H, NT], f16, tag="s3a", name=f"s3a_{b}")
                nc.vector.memset(s3a[:], 0.0)
                # s4[-1] := 0 (its -I contribution at t=0 must vanish; the
                # b4 bias row still fires, initializing v4 to b4)
                nc.gpsimd.memset(s4b[2][0:KC, :], 0.0)
                cp1 = cpp.tile([KC, CW], f16, tag="cp1", name=f"cp1_{b}")
                nc.gpsimd.memset(cp1[:], 1.0)

                def emit_L1(t, s0):
                    """15 weight matmuls of layer 1 (hi/lo x 7 chunks +
                    bias row on chunk 0)."""
                    for c in range(CH):
                        kc = KC + 1 if c == 0 else KC
                        rhs = s0[0:kc, c * NT:(c + 1) * NT]
                        nc.tensor.matmul(
                            v1, w1sh[0:kc, c * H:(c + 1) * H], rhs,
                            start=(t == 0 and c == 0), stop=False,
                            skip_group_check=True)
                        nc.tensor.matmul(
                            v1, w1sl[0:kc, c * H:(c + 1) * H], rhs,
                            start=False,
                            stop=(t == T - 1 and c == CH - 1),
                            skip_group_check=True)

                def emit_rst4(t, s4_prev, chunks):
                    for c in chunks:
                        dst = pv4[0:KC, c * NT:(c + 1) * NT]
                        # one start=True per psum bank (chunks pair 2-per-
                        # bank; odd chunks open implicitly - start=True
                        # clears has_written for the WHOLE bank)
                        nc.tensor.matmul(
                            dst, rst4[:, c * KC:(c + 1) * KC],
                            s4_prev[:, c * NT:(c + 1) * NT],
                            start=(t == 0 and c % 2 == 0), stop=False,
                            skip_group_check=True)

                # Software-pipelined emission: L1 of step t is emitted during
                # step t-1 so each engine's queue always has ready work to
                # fill the spike-latency gaps (rst4 fills the s1 wait, the
                # immediate post-read resets fill the s2/s3 waits).
                enc(fTt, cp1, 0, s0b[0])
                emit_L1(0, s0b[0])

                for t in range(T):
                    s0n = s0b[(t + 1) % 3]
                    s4_prev = s4b[(t - 1) % 3]
                    s4 = s4b[t % 3]
                    # spike of layer 1 (L1(t) was emitted in step t-1)
                    s1 = shp.tile([H, NT], f16, tag="s1")
                    nc.scalar.activation(s1[:], v1, ActF.Sigmoid,
                                         bias=nk1[:], scale=KAPPA)
                    # fills PE while ACT computes s1
                    emit_rst4(t, s4_prev, (0, 1, 2))

                    # -------- layer 2 (v2 shares v1's bank: no start) ----
                    nc.tensor.matmul(v2, w2h[:], s1[:], start=False,
                                     stop=False, skip_group_check=True)
                    nc.tensor.matmul(v2, w2l[:], s1[:], start=False,
                                     stop=(t == T - 1), skip_group_check=True)
                    # reset v1 right after its spike was read; fills s2 wait
                    if t < T - 1:
                        nc.tensor.matmul(v1, negI[:], s1[:], start=False,
                                         stop=False, skip_group_check=True)
                    emit_rst4(t, s4_prev, (3, 4, 5, 6))
                    s2 = shp.tile([H, NT], f16, tag="s2")
                    nc.scalar.activation(s2[:], v2, ActF.Sigmoid,
                                         bias=thh[:, t:t + 1], scale=KAPPA)

                    # -------- layer 3 --------
                    nc.tensor.matmul(v3, w3h[:], s2[:], start=(t == 0),
                                     stop=False, skip_group_check=True)
                    nc.tensor.matmul(v3, w3l[:], s2[:], start=False,
                                     stop=(t == T - 1), skip_group_check=True)
                    if t < T - 1:
                        nc.tensor.matmul(v2, negI[:], s2[:], start=False,
                                         stop=False, skip_group_check=True)
                    s3 = shp.tile([H, NT], f16, tag="s3")
                    nc.scalar.activation(s3[:], v3, ActF.Sigmoid,
                                         bias=thh[:, T + t:T + t + 1], scale=KAPPA)

                    # encoder for next step (independent; fills DVE)
                    if t + 1 < T:
                        enc(fTt, cp1, t + 1, s0n)

                    # -------- layer 4 weight matmuls --------
                    for c in range(CH):
                        dst = pv4[0:KC, c * NT:(c + 1) * NT]
                        nc.tensor.matmul(
                            dst, w4Th[:, c * KC:(c + 1) * KC], s3[:],
                            start=False, stop=False, skip_group_check=True)
                        if W4LO:
                            nc.tensor.matmul(
                                dst, w4Tl[:, c * KC:(c + 1) * KC], s3[:],
                                start=False, stop=False, skip_group_check=True)
                    if t < T - 1:
                        nc.tensor.matmul(v3, negI[:], s3[:], start=False,
                                         stop=False, skip_group_check=True)
                    nc.scalar.activation(s4[0:KC, :], pv4[0:KC, 0:CW],
                                         ActF.Sigmoid, bias=nk1[0:KC, :],
                                         scale=KAPPA)
                    # S3 accumulation (fp16 exact, max 16)
                    nc.vector.tensor_tensor(s3a[:], s3a[:], s3[:], Alu.add)
                    # next step's layer 1 (fills the tail of this slot)
                    if t + 1 < T:
                        emit_L1(t + 1, s0n)

                # ---- finish tile: count4 = W4*S3 + 16*b4 - v4_final ----
                # final reset (s4[15]) must be applied to v4 first
                s4_last = s4b[(T - 1) % 3]
                out = outp.tile([KC, CW], f32, tag="out")
                for c in range(CH):
                    dst = pv4[0:KC, c * NT:(c + 1) * NT]
                    nc.tensor.matmul(
                        dst, rst4[:, c * KC:(c + 1) * KC],
                        s4_last[:, c * NT:(c + 1) * NT],
                        start=False, stop=False, skip_group_check=True)
                    # note: this also adds one extra b4 (16 resets at t=1..15
                    # plus t=0 init plus this one = 17): compensated in b4v.
                    nc.tensor.matmul(
                        dst, nw4Th[:, c * KC:(c + 1) * KC], s3a[:],
                        start=False, stop=(not W4LO), skip_group_check=True)
                    if W4LO:
                        nc.tensor.matmul(
                            dst, nw4Tl[:, c * KC:(c + 1) * KC], s3a[:],
                            start=False, stop=True, skip_group_check=True)
                    # out = b4 - (v4 - W4*S3)/16 = count/16  (b4v = b4*(1+1/16)
                    # to cancel the extra bias-row add above)
                    nc.scalar.activation(
                        out[:, c * NT:(c + 1) * NT], dst, ActF.Identity,
                        bias=b4v[:, c:c + 1], scale=-1.0 / 16.0)
                for c in range(CH):
                    nc.sync.dma_start(
                        out_d[KC * c:KC * (c + 1), c0:c0 + NT],
                        out[:, c * NT:(c + 1) * NT])

    nc.compile()
    return nc


def _host_prep(inputs):
    f32 = np.float32
    f16 = np.float16
    features = np.asarray(inputs["features"], f32)
    fT = np.ascontiguousarray(features.T)  # [784, 16384]

    b1 = np.asarray(inputs["b1"], f32)
    b2 = np.asarray(inputs["b2"], f32)
    b3 = np.asarray(inputs["b3"], f32)
    b4 = np.asarray(inputs["b4"], f32)

    def split(w):
        hi = w.astype(f16)
        lo = (w - hi.astype(f32)).astype(f16)
        return hi, lo

    w1T = np.asarray(inputs["W1"], f32).T               # [784, 128]
    b1h, b1l = split(b1)
    w1Th, w1Tl = split(w1T)
    w1sh = np.zeros((KC + 1, CH * H), f16)
    w1sl = np.zeros((KC + 1, CH * H), f16)
    for c in range(CH):
        w1sh[0:KC, c * H:(c + 1) * H] = w1Th[c * KC:(c + 1) * KC, :]
        w1sl[0:KC, c * H:(c + 1) * H] = w1Tl[c * KC:(c + 1) * KC, :]
    w1sh[KC, 0:H] = b1h
    w1sl[KC, 0:H] = b1l

    w2h, w2l = split(np.ascontiguousarray(np.asarray(inputs["W2"], f32).T))
    w3h, w3l = split(np.ascontiguousarray(np.asarray(inputs["W3"], f32).T))
    w4Th, w4Tl = split(np.ascontiguousarray(np.asarray(inputs["W4"], f32).T))
    nw4Th = (-w4Th).astype(f16)
    nw4Tl = (-w4Tl).astype(f16)

    rst4 = np.zeros((KC + 1, IN), f16)
    for c in range(CH):
        blk = np.zeros((KC + 1, KC), f16)
        blk[0:KC, :] = -np.eye(KC, dtype=f16)
        blk[KC, :] = b4[c * KC:(c + 1) * KC].astype(f16)
        rst4[:, c * KC:(c + 1) * KC] = blk

    negI = (-np.eye(H)).astype(f16)

    # thresholds for layers 2,3: th[o,t] = 1 - cumsum(b)[t+1 terms], as
    # -KAPPA*th for the sigmoid bias. Iterative fp32 cumsum. v2 at step t
    # (0-based) has had (t+1) bias adds folded out.
    def cum_th(bv):
        c = np.zeros_like(bv)
        th = np.empty((bv.shape[0], T), f32)
        for t in range(T):
            c = (c + bv.astype(f32)).astype(f32)
            th[:, t] = (np.float32(1.0) - c).astype(f32)
        return th

    # layers 2/3 have no bias row: biases enter via time-varying threshold.
    # fp16 bias values to match what a bias row would have added? No - the
    # threshold path adds *exact* fp32 cumsum of fp32 b; reference adds b
    # each step in fp32. Close enough (tolerance 2e-2).
    thh = np.concatenate([cum_th(b2), cum_th(b3)], axis=1)  # [128, 32]
    nkthh = (-KAPPA * thh).astype(f32)

    # b4v: out = b4v - q/16 where q = v4T - W4*S3 and v4T includes 17
    # fp16-b4 bias-row adds (t=0 init + 15 per-step resets + final reset).
    # count = W4*S3 + 17*b4f16 - v4T_nobias... carefully:
    #   v4T = sum_t W4 s3_t + 17*b4f16 - sum_t s4_t
    #   => count = sum s4 = W4*S3 + 17*b4f16 - v4T
    #   out = count/16 = (17/16)*b4f16 - (v4T - W4*S3)/16
    b4f = b4.astype(f16).astype(f32)
    b4v = np.zeros((KC, CH), f32)
    for c in range(CH):
        b4v[:, c] = (17.0 / 16.0) * b4f[c * KC:(c + 1) * KC]

    shared = {
        "w1sh": np.ascontiguousarray(w1sh),
        "w1sl": np.ascontiguousarray(w1sl),
        "w2h": w2h, "w2l": w2l, "w3h": w3h, "w3l": w3l,
        "w4Th": w4Th, "w4Tl": w4Tl, "nw4Th": nw4Th, "nw4Tl": nw4Tl,
        "rst4": np.ascontiguousarray(rst4),
        "negI": negI,
        "thh": np.ascontiguousarray(nkthh),
        "b4v": np.ascontiguousarray(b4v),
    }
    in_maps = []
    for i in range(NCORES):
        m = dict(shared)
        m["fT"] = np.ascontiguousarray(fT[:, i * BC:(i + 1) * BC])
        in_maps.append(m)
    return in_maps


def kernel(**inputs):
    global LAST_RESULT
    if os.environ.get("BASS_TRACE"):
        _install_ntff_shim()
    from concourse.bass_utils import run_bass_kernel_spmd

    if "nc" not in _CACHE:
        _CACHE["nc"] = _build()
    nc = _CACHE["nc"]

    in_maps = _host_prep(inputs)
    kwargs = {}
    if os.environ.get("BASS_TRACE"):
        kwargs["tmpdir"] = os.environ.get("BASS_TRACE_DIR") or None
    try:
        res = run_bass_kernel_spmd(nc, in_maps, core_ids=list(range(NCORES)), **kwargs)
    except Exception:
        # transient device faults usually clear on retry
        import time

        time.sleep(2)
        res = run_bass_kernel_spmd(nc, in_maps, core_ids=list(range(NCORES)), **kwargs)
    LAST_RESULT = res

    outT = np.concatenate([res.results[i]["outT"] for i in range(NCORES)], axis=1)
    return np.ascontiguousarray(outT.T).astype(np.float32)
